# revision 7
# baseline (speedup 1.0000x reference)
"""ChebConv (K=3, 4 layers) GNN message passing on 8 Trainium2 NeuronCores.

Strategy:
  - Nodes sharded 8 ways by destination range (graph parallel).
  - Per layer l: out = h@(W0-W2) + L(h@W1 + 2 L(h@W2)) with L = -D^-1/2 A D^-1/2.
    The edge weight factorizes: w_e = -dis[src]*dis[dst], so the sparse op is a
    pure (unweighted) segment-sum of pre-scaled table rows:
       table = -dis * (h @ W2)    (per-node row scale, folded into evacuation)
       q[d]  = dis[d] * sum_{e: dst=d} table[src(e)]
  - Segment-sum on device: edges dst-sorted into 128-dst windows, chunks of 128
    edges; gather source rows via indirect DMA (128 rows / call); scatter via
    one-hot (indicator) matmul accumulating in PSUM.
  - Cross-core exchange of tables via AllGather collectives (bf16).
  - kernel() takes full inputs, shards internally, returns full output.
"""

import numpy as np
import ml_dtypes

import concourse.bass as bass
import concourse.mybir as mybir
from concourse.bass_utils import run_bass_kernel_spmd
from concourse.tile import TileContext

BF16 = ml_dtypes.bfloat16
P = 128
NCORE = 8


def _split_waits(nc, max_waits=1):
    """This walrus build accepts at most one semaphore wait per instruction;
    hoist extra waits onto preceding nops on the same engine."""
    for f in nc.m.functions:
        for blk in f.blocks:
            new_insts = []
            for inst in blk.instructions:
                si = getattr(inst, "sync_info", None)
                ow = list(si.on_wait) if si is not None and si.on_wait else []
                if len(ow) > max_waits:
                    chunks = [ow[i:i + max_waits] for i in range(0, len(ow), max_waits)]
                    for chunk in chunks[:-1]:
                        new_insts.append(mybir.InstNoOp(
                            name=nc.get_next_instruction_name(), ins=[], outs=[],
                            engine=inst.engine,
                            sync_info=mybir.SyncInfo(on_wait=list(chunk), on_update=[])))
                    si.on_wait = list(chunks[-1])
                new_insts.append(inst)
            blk.instructions = new_insts


def _prep_edges(src, dst, slice_n, nw):
    """Per-core dst-sorted edge arrays with SPMD-identical chunk structure."""
    core_of = dst // slice_n
    counts = np.zeros((NCORE, nw), np.int64)
    per_core = []
    for c in range(NCORE):
        m = core_of == c
        s_c, d_c = src[m], dst[m] - c * slice_n
        order = np.argsort(d_c, kind="stable")
        s_c, d_c = s_c[order], d_c[order]
        w_c = d_c >> 7
        counts[c] = np.bincount(w_c, minlength=nw)
        per_core.append((s_c, d_c, w_c))
    chunks_w = np.maximum(1, np.ceil(counts.max(axis=0) / P)).astype(np.int64)
    tot_chunks = int(chunks_w.sum())
    slot_of_w = np.concatenate([[0], np.cumsum(chunks_w)])[:-1]

    idx_arr = np.zeros((NCORE, P, tot_chunks), np.int32)
    dst_arr = np.full((NCORE, P, tot_chunks), 128.0, np.float32)  # 128 => pad lane
    for c in range(NCORE):
        s_c, d_c, w_c = per_core[c]
        win_start = np.concatenate([[0], np.cumsum(counts[c])])[:-1]
        pos_in_w = np.arange(len(d_c)) - win_start[w_c]
        chunk = slot_of_w[w_c] + (pos_in_w >> 7)
        lane = pos_in_w & 127
        idx_arr[c, lane, chunk] = s_c
        dst_arr[c, lane, chunk] = (d_c & 127).astype(np.float32)
    return idx_arr, dst_arr, [int(v) for v in chunks_w], [int(v) for v in slot_of_w]


def kernel(x, edge_index, W1, b1, W2, b2, W3, b3, W4, b4):
    x = np.asarray(x, np.float32)
    edge_index_in = edge_index
    edge_index = np.asarray(edge_index)
    n_nodes, f_in = x.shape
    assert n_nodes % NCORE == 0
    slice_n = n_nodes // NCORE
    nw = (slice_n + P - 1) // P
    last_rows = slice_n - (nw - 1) * P
    H = W1.shape[2]
    C = W4.shape[2]
    C_PAD = 16

    src = np.asarray(edge_index[0]).astype(np.int32)
    dst = np.asarray(edge_index[1]).astype(np.int32)

    deg = np.bincount(dst, minlength=n_nodes).astype(np.float32)
    dis = np.where(deg > 0, 1.0 / np.sqrt(np.maximum(deg, 1e-12)), 0.0).astype(np.float32)

    idx_arr, dstloc_arr, chunks_w, slot_of_w = _prep_edges(src, dst, slice_n, nw)
    tot_chunks = idx_arr.shape[2]

    node_grid = (np.arange(NCORE)[:, None, None] * slice_n
                 + np.arange(nw)[None, None, :] * P
                 + np.arange(P)[None, :, None])
    node_grid = np.minimum(node_grid, n_nodes - 1)
    dis_cols = dis[node_grid].astype(np.float32)
    negdis_cols = (-dis[node_grid]).astype(np.float32)
    c2_cols = (-2.0 * dis[node_grid] ** 2).astype(np.float32)

    def wprep(W, b):
        A = np.asarray(W[0], np.float32) - np.asarray(W[2], np.float32)
        return (A.astype(BF16), np.asarray(W[1], np.float32).astype(BF16),
                np.asarray(W[2], np.float32).astype(BF16),
                np.asarray(b, np.float32))
    A1, W1b, W1c, b1f = wprep(W1, b1)
    A2, W2b, W2c, b2f = wprep(W2, b2)
    A3, W3b, W3c, b3f = wprep(W3, b3)
    A4, W4b, W4c, b4f = wprep(W4, b4)
    w_arrs = [("A1", A1), ("W1b", W1b), ("W1c", W1c),
              ("A2", A2), ("W2b", W2b), ("W2c", W2c),
              ("A3", A3), ("W3b", W3b), ("W3c", W3c),
              ("A4", A4), ("W4b", W4b), ("W4c", W4c)]

    nodes_pad = nw * P
    xT = np.ascontiguousarray(x.T.astype(BF16))
    xT_pad = np.zeros((NCORE, f_in, nodes_pad), BF16)
    for c in range(NCORE):
        xT_pad[c, :, :slice_n] = xT[:, c * slice_n:(c + 1) * slice_n]

    iota = np.broadcast_to(np.arange(P, dtype=np.float32), (P, P)).copy()

    def bias_rep(b, n):
        return np.broadcast_to(np.asarray(b, np.float32), (P, n)).copy()

    # ---------------- device program ----------------
    nc = bass.Bass(num_devices=NCORE)
    dt = mybir.dt

    t_xt = nc.dram_tensor("xt", [f_in, nodes_pad], dt.bfloat16, kind="ExternalInput")
    t_idx = nc.dram_tensor("idx", [P, tot_chunks], dt.int32, kind="ExternalInput")
    t_dstloc = nc.dram_tensor("dstloc", [P, tot_chunks], dt.float32, kind="ExternalInput")
    t_iota = nc.dram_tensor("iota", [P, P], dt.float32, kind="ExternalInput")
    t_dis = nc.dram_tensor("dis", [P, nw], dt.float32, kind="ExternalInput")
    t_negdis = nc.dram_tensor("negdis", [P, nw], dt.float32, kind="ExternalInput")
    t_c2 = nc.dram_tensor("c2", [P, nw], dt.float32, kind="ExternalInput")
    t_W = {nm: nc.dram_tensor(nm, list(arr.shape), dt.bfloat16, kind="ExternalInput")
           for nm, arr in w_arrs}
    t_b = {f"bias{i}": nc.dram_tensor(f"bias{i}", [P, H if i < 4 else C], dt.float32,
                                      kind="ExternalInput") for i in (1, 2, 3, 4)}

    t_y = nc.dram_tensor("y", [slice_n, C], dt.float32, kind="ExternalOutput")
    t_hbuf = nc.dram_tensor("hbuf", [nodes_pad, H], dt.bfloat16, kind="Internal")

    cc = {}
    for l in (1, 2, 3):
        for z in ("A", "B"):
            cc[(l, z + "_in")] = nc.dram_tensor(f"cc{z}i{l}", [slice_n, H], dt.bfloat16,
                                                kind="Internal")
            cc[(l, z + "_out")] = nc.dram_tensor(f"cc{z}o{l}", [n_nodes, H], dt.bfloat16,
                                                 kind="Internal", addr_space="Shared")
    for z in ("A", "B"):
        cc[(4, z + "_in")] = nc.dram_tensor(f"cc{z}i4", [slice_n, C_PAD], dt.bfloat16,
                                            kind="Internal")
        cc[(4, z + "_out")] = nc.dram_tensor(f"cc{z}o4", [n_nodes, C_PAD], dt.bfloat16,
                                             kind="Internal", addr_space="Shared")

    RG = [list(range(NCORE))]
    WPB = 14
    batches = [list(range(b, min(b + WPB, nw))) for b in range(0, nw, WPB)]
    max_slots = max(sum(chunks_w[w] for w in ws) for ws in batches)

    with TileContext(nc) as tc:
        with tc.tile_pool(name="sb", bufs=1) as sb, \
             tc.tile_pool(name="gp", bufs=2) as gpool, \
             tc.tile_pool(name="ohp", bufs=4) as ohpool, \
             tc.tile_pool(name="evp", bufs=4) as evpool, \
             tc.tile_pool(name="psp", bufs=1, space="PSUM") as ps_proj, \
             tc.tile_pool(name="pss", bufs=4, space="PSUM") as ps_spmv:

            iota_t = sb.tile([P, P], dt.float32)
            nc.sync.dma_start(out=iota_t[:], in_=t_iota[:])
            idx_t = sb.tile([P, tot_chunks], dt.int32)
            nc.sync.dma_start(out=idx_t[:], in_=t_idx[:])
            dstloc_t = sb.tile([P, tot_chunks], dt.float32)
            nc.sync.dma_start(out=dstloc_t[:], in_=t_dstloc[:])
            dis_t = sb.tile([P, nw], dt.float32)
            nc.sync.dma_start(out=dis_t[:], in_=t_dis[:])
            negdis_t = sb.tile([P, nw], dt.float32)
            nc.sync.dma_start(out=negdis_t[:], in_=t_negdis[:])
            c2_t = sb.tile([P, nw], dt.float32)
            nc.sync.dma_start(out=c2_t[:], in_=t_c2[:])

            w_t = {}
            for nm, arr in w_arrs:
                kdim = arr.shape[0]
                w_t[nm] = sb.tile([min(kdim, P), arr.shape[1]], dt.bfloat16,
                                  tag=f"w_{nm}", name=f"w_{nm}")
                if kdim <= P:
                    nc.sync.dma_start(out=w_t[nm][:], in_=t_W[nm][:])
                else:
                    nc.sync.dma_start(out=w_t[nm][:], in_=t_W[nm][:P])
                    w_t[nm + "_hi"] = sb.tile([kdim - P, arr.shape[1]], dt.bfloat16,
                                              tag=f"w_{nm}h", name=f"w_{nm}h")
                    nc.sync.dma_start(out=w_t[nm + "_hi"][:], in_=t_W[nm][P:])
            b_t = {}
            for i in (1, 2, 3, 4):
                ncols = H if i < 4 else C
                b_t[i] = sb.tile([P, ncols], dt.float32, tag=f"b_{i}", name=f"b_{i}")
                nc.sync.dma_start(out=b_t[i][:], in_=t_b[f"bias{i}"][:])

            p0b_store = sb.tile([P, nw * H], dt.bfloat16)
            p1_store = sb.tile([P, nw * H], dt.bfloat16)
            p0b4_store = sb.tile([P, nw * C], dt.float32)
            p14_store = sb.tile([P, nw * C_PAD], dt.bfloat16)
            nc.vector.memset(p14_store[:], 0.0)

            def projections(layer, hT_tiles):
                if layer == 4:
                    An, Wb, Wc = "A4", "W4b", "W4c"
                    ncols, st0, st1, stw1 = C, p0b4_store, p14_store, C_PAD
                else:
                    An, Wb, Wc = f"A{layer}", f"W{layer}b", f"W{layer}c"
                    ncols, st0, st1, stw1 = H, p0b_store, p1_store, H
                ccA = cc[(layer, "A_in")]
                for w in range(nw):
                    rows = last_rows if w == nw - 1 else P
                    psA = ps_proj.tile([P, ncols], dt.float32, space="PSUM", tag="psA")
                    psB = ps_proj.tile([P, ncols], dt.float32, space="PSUM", tag="psB")
                    psC = ps_proj.tile([P, ncols], dt.float32, space="PSUM", tag="psC")
                    for ki, ht in enumerate(hT_tiles):
                        lhs = ht[:, w * P:(w + 1) * P]
                        sfx = "_hi" if ki else ""
                        first, last = ki == 0, ki == len(hT_tiles) - 1
                        nc.tensor.matmul(out=psA[:], lhsT=lhs, rhs=w_t[An + sfx][:],
                                         start=first, stop=last)
                        nc.tensor.matmul(out=psB[:], lhsT=lhs, rhs=w_t[Wb + sfx][:],
                                         start=first, stop=last)
                        nc.tensor.matmul(out=psC[:], lhsT=lhs, rhs=w_t[Wc + sfx][:],
                                         start=first, stop=last)
                    nc.vector.tensor_tensor(out=st0[:, w * ncols:(w + 1) * ncols],
                                            in0=psA[:], in1=b_t[layer][:],
                                            op=mybir.AluOpType.add)
                    nc.vector.tensor_scalar(out=st1[:, w * stw1: w * stw1 + ncols],
                                            in0=psB[:], scalar1=negdis_t[:, w:w + 1],
                                            scalar2=None, op0=mybir.AluOpType.mult)
                    ev = evpool.tile([P, C_PAD if layer == 4 else ncols], dt.bfloat16,
                                     tag="evP2")
                    if layer == 4:
                        nc.vector.memset(ev[:], 0.0)
                        nc.vector.tensor_scalar(out=ev[:, :C], in0=psC[:],
                                                scalar1=negdis_t[:, w:w + 1], scalar2=None,
                                                op0=mybir.AluOpType.mult)
                    else:
                        nc.vector.tensor_scalar(out=ev[:], in0=psC[:],
                                                scalar1=negdis_t[:, w:w + 1], scalar2=None,
                                                op0=mybir.AluOpType.mult)
                    nc.sync.dma_start(out=ccA[w * P: w * P + rows, :], in_=ev[:rows, :])

            def spmv(table, gcols, out_cb):
                for ws in batches:
                    g = gpool.tile([P, max_slots * gcols], dt.bfloat16, tag="G")
                    pos = 0
                    for w in ws:
                        for k in range(chunks_w[w]):
                            slot = slot_of_w[w] + k
                            nc.gpsimd.indirect_dma_start(
                                out=g[:, pos * gcols:(pos + 1) * gcols],
                                out_offset=None, in_=table[:],
                                in_offset=bass.IndirectOffsetOnAxis(
                                    ap=idx_t[:, slot:slot + 1], axis=0))
                            pos += 1
                    pos = 0
                    for w in ws:
                        psq = ps_spmv.tile([P, gcols], dt.float32, space="PSUM", tag="psq")
                        for k in range(chunks_w[w]):
                            slot = slot_of_w[w] + k
                            oh = ohpool.tile([P, P], dt.bfloat16, tag="oh")
                            nc.vector.tensor_scalar(
                                out=oh[:], in0=iota_t[:],
                                scalar1=dstloc_t[:, slot:slot + 1], scalar2=None,
                                op0=mybir.AluOpType.is_equal)
                            nc.tensor.matmul(out=psq[:], lhsT=oh[:],
                                             rhs=g[:, pos * gcols:(pos + 1) * gcols],
                                             start=(k == 0), stop=(k == chunks_w[w] - 1))
                            pos += 1
                        out_cb(w, psq)

            def spmv1_out(layer):
                ncols = H if layer < 4 else C_PAD
                ccB = cc[(layer, "B_in")]
                st1 = p1_store if layer < 4 else p14_store

                def cb(w, psq):
                    rows = last_rows if w == nw - 1 else P
                    tmp = evpool.tile([P, ncols], dt.bfloat16, tag="ev1tmp")
                    u = evpool.tile([P, ncols], dt.bfloat16, tag="ev1u")
                    nc.vector.tensor_scalar(out=tmp[:], in0=psq[:],
                                            scalar1=c2_t[:, w:w + 1], scalar2=None,
                                            op0=mybir.AluOpType.mult)
                    nc.vector.tensor_tensor(out=u[:], in0=tmp[:],
                                            in1=st1[:, w * ncols:(w + 1) * ncols],
                                            op=mybir.AluOpType.add)
                    nc.sync.dma_start(out=ccB[w * P: w * P + rows, :], in_=u[:rows, :])
                return cb

            def spmv2_out(layer):
                def cb(w, psq):
                    rows = last_rows if w == nw - 1 else P
                    if layer < 4:
                        tmp = evpool.tile([P, H], dt.bfloat16, tag="ev2tmp")
                        o = evpool.tile([P, H], dt.bfloat16, tag="ev2o")
                        ob = evpool.tile([P, H], dt.bfloat16, tag="ev2ob")
                        nc.vector.tensor_scalar(out=tmp[:], in0=psq[:],
                                                scalar1=dis_t[:, w:w + 1], scalar2=None,
                                                op0=mybir.AluOpType.mult)
                        nc.vector.tensor_tensor(out=o[:], in0=tmp[:],
                                                in1=p0b_store[:, w * H:(w + 1) * H],
                                                op=mybir.AluOpType.add)
                        nc.vector.tensor_scalar(out=ob[:], in0=o[:],
                                                scalar1=6.0, scalar2=0.0,
                                                op0=mybir.AluOpType.min,
                                                op1=mybir.AluOpType.max)
                        nc.sync.dma_start(out=t_hbuf[w * P:(w + 1) * P, :], in_=ob[:, :])
                    else:
                        tmp = evpool.tile([P, C], dt.float32, tag="ev4tmp")
                        o = evpool.tile([P, C], dt.float32, tag="ev4o")
                        nc.vector.tensor_scalar(out=tmp[:], in0=psq[:, :C],
                                                scalar1=dis_t[:, w:w + 1], scalar2=None,
                                                op0=mybir.AluOpType.mult)
                        nc.vector.tensor_tensor(out=o[:], in0=tmp[:],
                                                in1=p0b4_store[:, w * C:(w + 1) * C],
                                                op=mybir.AluOpType.add)
                        nc.sync.dma_start(out=t_y[w * P: w * P + rows, :], in_=o[:rows, :])
                return cb

            def allgather(l, z):
                nc.gpsimd.collective_compute(
                    "AllGather", mybir.AluOpType.bypass,
                    ins=[cc[(l, z + "_in")][:]], outs=[cc[(l, z + "_out")][:]],
                    replica_groups=RG)

            xt0 = sb.tile([P, nodes_pad], dt.bfloat16, tag="hfeat0")
            nc.sync.dma_start(out=xt0[:], in_=t_xt[:P, :])
            hT_tiles = [xt0]
            if f_in > P:
                xt1 = sb.tile([f_in - P, nodes_pad], dt.bfloat16, tag="hfeat_hi")
                nc.sync.dma_start(out=xt1[:], in_=t_xt[P:, :])
                hT_tiles.append(xt1)

            import os as _os
            _nl = int(_os.environ.get("KERNEL_NLAYERS", "4"))
            _nstage = int(_os.environ.get("KERNEL_NSTAGE", "5"))
            for layer in (1, 2, 3, 4)[:_nl]:
                st = _nstage if layer == _nl else 5
                gcols = H if layer < 4 else C_PAD
                projections(layer, hT_tiles)
                if st >= 2:
                    allgather(layer, "A")
                if st >= 3:
                    spmv(cc[(layer, "A_out")], gcols, spmv1_out(layer))
                if st >= 4:
                    allgather(layer, "B")
                if st >= 5:
                    spmv(cc[(layer, "B_out")], gcols, spmv2_out(layer))
                if layer < 4 and st >= 5:
                    hT = sb.tile([P, nodes_pad], dt.bfloat16, tag="hfeat2")
                    nc.sync.dma_start(out=hT[:], in_=t_hbuf[:], transpose=True)
                    hT_tiles = [hT]
            if _nl < 4 or _nstage < 5:
                dummy = evpool.tile([P, C], dt.float32, tag="dummyy")
                nc.vector.memset(dummy[:], 0.0)
                for w in range(nw):
                    rows = last_rows if w == nw - 1 else P
                    nc.sync.dma_start(out=t_y[w * P: w * P + rows, :], in_=dummy[:rows, :])

    # ---------------- run ----------------
    in_maps = []
    for c in range(NCORE):
        m = {
            "xt": np.asarray(xT_pad[c]),
            "idx": idx_arr[c], "dstloc": dstloc_arr[c], "iota": iota,
            "dis": dis_cols[c], "negdis": negdis_cols[c], "c2": c2_cols[c],
            "bias1": bias_rep(b1f, H), "bias2": bias_rep(b2f, H),
            "bias3": bias_rep(b3f, H), "bias4": bias_rep(b4f, C),
        }
        for nm, arr in w_arrs:
            m[nm] = np.asarray(arr)
        in_maps.append(m)

    import os
    if not os.environ.get("KERNEL_SIM"):
        _split_waits(nc)
    if os.environ.get("KERNEL_SIM"):
        from concourse.bass_interp import MultiCoreSim
        sim = MultiCoreSim(nc, num_cores=NCORE, num_workers=int(os.environ.get("KERNEL_SIM_WORKERS", "0")) or None)
        for c in range(NCORE):
            for k, v in in_maps[c].items():
                sim.cores[c].tensor(k)[:] = v
        sim.simulate()
        y = np.concatenate([np.asarray(sim.cores[c].tensor("y")) for c in range(NCORE)], axis=0)
        return (y, edge_index_in)

    import time
    t0 = time.time()
    res = run_bass_kernel_spmd(nc, in_maps, core_ids=list(range(NCORE)))
    global LAST_RUN_NS
    LAST_RUN_NS = int((time.time() - t0) * 1e9)  # compile+dispatch+exec wall
    y = np.concatenate([res.results[c]["y"] for c in range(NCORE)], axis=0)
    return (y, edge_index_in)


LAST_RUN_NS = -1
